# revision 4
# baseline (speedup 1.0000x reference)
"""HCMaskLayer region-mean kernel for Trainium2 (8 NeuronCores).

Math: the reference computes a 2D summed-area table of image [2048,2048,64]
and takes per-region rectangle means.  Equivalently, for region r and
channel c:

    sums[r, c] = sum_{i,j} w[i, r] * v[j, r] * image[i, j, c]

with w[i, r] = [i < x1_r] - [i < x0_r] and v[j, r] = [j < y1_r] - [j < y0_r]
(identical to the SAT corner-difference formula, for arbitrary indices).

Implementation: one streaming pass over the image on the TensorEngine.
The fp32 image is quantized on the host to a SINGLE fp8_e4m3 stream
(1 byte/elem, 1/4 the fp32 DMA traffic) using error diffusion along j:
    q[j] = e4m3(x[j] + carry);  carry += x[j] - q[j]
Because every region sum contracts a contiguous j-interval, the interval
sum error telescopes to carry(y0)-carry(y1) (~1 quantization step per
row) instead of growing like sqrt(len).  Measured end-to-end rel err vs
the reference: ~2e-3 (plain e4m3 rounding would be ~2.7e-2).

Each core takes a 256-row slab, host-packed so that each (partition, batch)
is one contiguous 8 KB run in HBM:
  - partition dim = j (mod 128); matmuls use fp8 DoubleRow perf mode to
    contract 256 j-values per instruction (2 fp8 weights per PE cell),
  - for each 8-row batch, 8 DoubleRow matmuls contract j against the
    column mask V, accumulating G[r, i, c] in one PSUM bank,
  - VectorEngine multiplies by the row mask w[i, r] and accumulates into
    a [64, 8, 64] buffer; a single strided reduce at the end folds i.
Host sums the 8 per-core partials and applies the count division/guard.
"""

import sys
import types

import numpy as np
import ml_dtypes


def _ensure_axon_hooks():
    """bass_utils imports antenv.axon_hooks when BASS_TRACE=1 under axon;
    provide a stub registry if the image lacks that module."""
    try:
        import antenv.axon_hooks  # noqa: F401
    except ImportError:
        try:
            import antenv
        except ImportError:
            return
        mod = types.ModuleType("antenv.axon_hooks")
        mod._hook = None
        mod.set_axon_ntff_profile_hook = lambda h: setattr(mod, "_hook", h)
        mod.get_axon_ntff_profile_hook = lambda: mod._hook
        sys.modules["antenv.axon_hooks"] = mod
        antenv.axon_hooks = mod
    # The axon boot hook registration runs at interpreter start, BEFORE this
    # stub module exists, so it degrades silently and tracing is skipped.
    # Re-register the NTFF profile hook here if none is present.
    try:
        import antenv.axon_hooks as ah

        if ah.get_axon_ntff_profile_hook() is None:
            from trn_agent_boot.trn_boot import _ntff_profile_via_ctypes

            hook = _ntff_profile_via_ctypes("/opt/axon/libaxon_pjrt.so")
            if hook is not None:
                ah.set_axon_ntff_profile_hook(hook)
    except Exception:
        pass


_ensure_axon_hooks()

N = 2048          # image height/width
C = 64            # channels
R = 64            # regions
NCORES = 8
SLAB = N // NCORES  # 256 rows per core
BI = 8            # rows per batch (PSUM free = BI*C = 512 fp32 = 1 bank)
NB = SLAB // BI   # 32 batches per core
Q = 8             # DoubleRow matmuls per batch (each contracts 256 j)
KB = 4            # batches per DMA chunk (4.2 MB transfers)
NCHUNK = NB // KB

_CACHED = {}


def _build_nc():
    import concourse.mybir as mybir
    import concourse.tile as tile
    from concourse import bacc

    nc = bacc.Bacc("TRN2", target_bir_lowering=False, debug=False,
                   num_devices=NCORES)
    bf16 = mybir.dt.bfloat16
    fp8 = mybir.dt.float8e4
    f32 = mybir.dt.float32

    img = nc.dram_tensor("img", [128, NB, Q, 2, BI, C], fp8,
                         kind="ExternalInput")
    vt = nc.dram_tensor("vt", [128, Q, 2, R], fp8, kind="ExternalInput")
    wb = nc.dram_tensor("wb", [R, NB, BI, C], bf16, kind="ExternalInput")
    out = nc.dram_tensor("partial", [R, C], f32, kind="ExternalOutput")

    with tile.TileContext(nc) as tc:
        with (
            tc.tile_pool(name="const", bufs=1) as const_pool,
            tc.tile_pool(name="loads", bufs=3) as loads,
            tc.tile_pool(name="psum", bufs=2, space="PSUM") as psum_pool,
            tc.tile_pool(name="temps", bufs=3) as temps,
        ):
            vt_s = const_pool.tile([128, Q, 2, R], fp8)
            nc.sync.dma_start(out=vt_s[:], in_=vt[:])
            wb_s = const_pool.tile([R, NB, BI, C], bf16)
            nc.sync.dma_start(out=wb_s[:], in_=wb[:])
            accb = const_pool.tile([R, BI, C], f32)
            nc.vector.memset(accb[:], 0.0)

            for ch in range(NCHUNK):
                img_t = loads.tile([128, KB, Q, 2, BI, C], fp8, tag="img")
                nc.sync.dma_start(out=img_t[:], in_=img[:, ch * KB:(ch + 1) * KB])
                for bb in range(KB):
                    b = ch * KB + bb
                    g = psum_pool.tile([R, BI, C], f32, tag="g")
                    for q in range(Q):
                        nc.tensor.matmul(
                            g[:], lhsT=vt_s[:, q], rhs=img_t[:, bb, q],
                            start=(q == 0), stop=(q == Q - 1),
                            perf_mode=mybir.MatmulPerfMode.DoubleRow)
                    tmp = temps.tile([R, BI, C], f32, tag="tmp")
                    nc.vector.tensor_mul(tmp[:], g[:], wb_s[:, b])
                    nc.vector.tensor_add(accb[:], accb[:], tmp[:])

            red = const_pool.tile([R, C], f32)
            nc.vector.reduce_sum(
                red[:], accb.rearrange("r i c -> r c i"),
                axis=mybir.AxisListType.X)
            nc.sync.dma_start(out=out[:], in_=red[:])
    nc.compile()
    return nc


def _get_nc():
    if "nc" not in _CACHED:
        _CACHED["nc"] = _build_nc()
    return _CACHED["nc"]


def _quantize_diffuse(image):
    """fp8_e4m3 quantization with error diffusion along axis 1 (j).

    Processed in j-blocks with a vectorized scan over (i, c) lanes."""
    q = np.empty(image.shape, dtype=ml_dtypes.float8_e4m3)
    carry = np.zeros((image.shape[0], image.shape[2]), dtype=np.float32)
    for j in range(image.shape[1]):
        v = image[:, j, :] + carry
        qj = v.astype(ml_dtypes.float8_e4m3)
        q[:, j, :] = qj
        carry = v - qj.astype(np.float32)
    return q


def _pack(slab):
    """[SLAB, N, C] fp8 -> [128, NB, Q, 2, BI, C]:
    out[p, b, q, i2, ii, c] = slab[b*BI+ii, q*256 + i2*128 + p, c]."""
    x = slab.reshape(NB, BI, Q, 2, 128, C)
    return np.ascontiguousarray(x.transpose(4, 0, 2, 3, 1, 5))


def kernel(image, x0, x1, y0, y1):
    from concourse.bass_utils import run_bass_kernel_spmd

    # Initialize the axon PJRT plugin up-front: its boot hook registers the
    # NTFF profile hook into antenv.axon_hooks, which run_bass_kernel_spmd
    # checks BEFORE first jax use — without this, tracing is silently skipped.
    import jax

    jax.devices()

    image = np.ascontiguousarray(np.asarray(image, dtype=np.float32))
    x0 = np.asarray(x0).astype(np.int64)
    x1 = np.asarray(x1).astype(np.int64)
    y0 = np.asarray(y0).astype(np.int64)
    y1 = np.asarray(y1).astype(np.int64)

    idx = np.arange(N, dtype=np.int64)[:, None]
    # +-1/0 interval masks; exactly the SAT corner-difference weights
    W = (idx < x1[None, :]).astype(np.float32) - (idx < x0[None, :]).astype(np.float32)
    V = (idx < y1[None, :]).astype(np.float32) - (idx < y0[None, :]).astype(np.float32)

    q8 = _quantize_diffuse(image)

    vt_m = np.ascontiguousarray(
        V.reshape(Q, 2, 128, R).transpose(2, 0, 1, 3).astype(ml_dtypes.float8_e4m3))

    in_maps = []
    for m in range(NCORES):
        sl = slice(m * SLAB, (m + 1) * SLAB)
        wslab = W[sl]                                # [SLAB, R]
        wbm = np.ascontiguousarray(
            np.broadcast_to(wslab.T[:, :, None], (R, SLAB, C)).astype(ml_dtypes.bfloat16)
        ).reshape(R, NB, BI, C)
        in_maps.append({
            "img": _pack(q8[sl]),
            "vt": vt_m,
            "wb": wbm,
        })

    res = run_bass_kernel_spmd(_get_nc(), in_maps, core_ids=list(range(NCORES)))
    _CACHED["last_result"] = res

    sums = np.zeros((R, C), dtype=np.float32)
    for r in res.results:
        sums += r["partial"]

    cnt = ((x1 - x0) * (y1 - y0)).astype(np.float32)
    denom = np.maximum(cnt, 1.0).astype(np.float32)
    outv = np.where(cnt[:, None] > 0, sums / denom[:, None],
                    np.float32(0.0)).astype(np.float32)
    return outv
